# revision 3
# baseline (speedup 1.0000x reference)
"""Trainium2 Bass kernel v4 for nn_FDDiscriminator (batched RBF-Gram MMD).

Math (matches reference): see baseline docstring.  254 time-pair slices
dedup to T=128 slices with weights w in {1,2}.  Per slice t:
  Kxy (512x512, weight c_y = w*2(B-1)/B folded into exponent shift),
  Kxx upper-tri via 6 off-diag 128-blocks (c=2w) + 4 full diagonal
  128-blocks (c=w; full = 2*upper + diag, so host subtracts 512*254).

Device strategy per core (16 slices):
  ALL grams via fp8e4 DoubleRow matmuls (0.5 PE cycles/col).  Operands
  use a hi+lo fp8 split (a = h+l exact to ~2^-8) with K_eff=76 rows:
  pairs (h,h'),(l,h'),(h,l'),(l,l') give the EXACT a~.b~ plus 6-term fp8
  norm rows and 6-term shifted-norm rows -> PSUM d to ~1e-2 abs.
  exp is split between ACT (exact, 1024-col windows) and DVE
  (Schraudolph bf16: int16(d*C1+C2) bit-pattern, 512-col windows).
  Diagonal blocks are forced into ACT windows: d_mm ~ -2ln(w) + eps so
  exp -> w exactly after bf16 rounding; host subtracts 512*254.
  Reduction: per-(slice,type) accumulation brackets of bf16 matmuls
  against a ones column into one PSUM red bank (2*SPT columns),
  emitted with a 2-slice lag.  PSUM banks: ACT 2x[1024] + DVE 3x[512]
  + red = 8.  Host: out = (C_xx - 512*254 - C_xy)/(B(B-1))/254.
"""

import numpy as np
import ml_dtypes

BF16 = ml_dtypes.bfloat16
F8 = ml_dtypes.float8_e4m3

B = 512
T = 128
C = 16
NCORES = 8
SPT = T // NCORES
KP = 38          # physical contraction rows (2 k-tiles of 38 -> K_eff 76)

WA = 1024        # ACT window capacity (cols)
WD = 512         # DVE window capacity

# Schraudolph (bf16 flavour): int16(d*C1B + C2B) bit-viewed as bf16 ~ exp(-d/2)
SIGMA = 0.0575
C1B = float(np.float32(-0.5 * np.log2(np.e) * (1 << 7)))
C2B = float(np.float32((127.0 - SIGMA) * (1 << 7)))

_CACHE = {}


def _plan():
    """Static per-core schedule.

    Returns (windows, brackets_at, n_windows) where windows is a list of
      dict(engine='A'|'D', segs=[(kind, s, i, j, n0, n1, coff), ...])
    kind: 'xy' (lhsT x-strip i vs y cols n0:n1), 'up' (x-strip i vs
    x-strip j cols n0:n1), 'dg' (x-strip i vs x-strip i, c=w).
    coff = column offset in the window.  brackets_at[w] = list of
    (slice, type) brackets to emit before window w's gram matmuls.
    """
    # Per-slice segment queue: dg first (ACT-forced), then xy, then up.
    # Each seg: (kind, s, i, j, width_total) - splittable at any multiple
    # of 64 cols.
    rate = {"A": 1.0 / 1.2, "D": 1.0417}
    ovh = {"A": 235.0, "D": 128.0}
    cap = {"A": WA, "D": WD}
    t = {"A": 0.0, "D": 0.0}
    windows = []
    # window content bookkeeping for reduce: runs[(s, type)] = list of
    # (window_idx, coff, width)
    runs = {}
    # build the global segment stream
    segs = []
    for s in range(SPT):
        # slice 0: xy first so the first window only needs the first half
        # of the (split) slice-0 input DMA
        if s == 0:
            for i in range(4):
                segs.append(["xy", s, i, 0, 512])
            for i in range(4):
                segs.append(["dg", s, i, i, 128])
        else:
            for i in range(4):
                segs.append(["dg", s, i, i, 128])
            for i in range(4):
                segs.append(["xy", s, i, 0, 512])
        k = 0
        for i in range(4):
            for j in range(i + 1, 4):
                segs.append(["up", s, i, j, 128])
                k += 1
    total_cols = sum(sg[4] for sg in segs)
    done_cols = 0
    si = 0
    while si < len(segs):
        # choose engine: earliest-free; dg segments require ACT
        e = "A" if t["A"] <= t["D"] else "D"
        if segs[si][0] == "dg" and e == "D":
            e = "A"
        w = {"engine": e, "segs": []}
        coff = 0
        room = cap[e]
        while si < len(segs) and room > 0:
            kind, s, i, j, width = segs[si]
            if kind == "dg" and e == "D":
                break
            # never let a single matmul cross a 2KB PSUM bank boundary
            take = min(width, room, 512 - (coff % 512))
            # n0: consumed-so-far within this seg's rhs range
            full = 512 if kind == "xy" else 128
            n0 = full - width
            w["segs"].append((kind, s, i, j, n0, n0 + take, coff))
            typ = "xy" if kind == "xy" else "xx"
            runs.setdefault((s, typ), []).append((len(windows), coff, take))
            coff += take
            room -= take
            done_cols += take
            if take == width:
                si += 1
            else:
                segs[si][4] = width - take
        t[e] += ovh[e] + coff * rate[e]
        windows.append(w)

    # brackets: (s, typ) emitted before the first window of slice s+2
    first_win_of_slice = {}
    for wi, w in enumerate(windows):
        for seg in w["segs"]:
            s = seg[1]
            first_win_of_slice.setdefault(s, wi)
    brackets_at = {}
    order = []
    for s in range(SPT):
        order.append((s, "xy"))
        order.append((s, "xx"))
    for (s, typ) in order:
        at = first_win_of_slice.get(s + 2, len(windows))
        brackets_at.setdefault(at, []).append((s, typ))
    return windows, brackets_at, runs


def _build_bass():
    import concourse.bass as bass
    import concourse.bacc as bacc
    import concourse.tile as tile
    import concourse.mybir as mybir

    f32 = mybir.dt.float32
    i16 = mybir.dt.int16
    bf16 = mybir.dt.bfloat16
    f8 = mybir.dt.float8e4
    u8 = mybir.dt.uint8
    Exp = mybir.ActivationFunctionType.Exp
    Mult = mybir.AluOpType.mult
    Add = mybir.AluOpType.add
    DR = mybir.MatmulPerfMode.DoubleRow

    nc = bacc.Bacc(
        "TRN2", target_bir_lowering=False, debug=False, num_devices=NCORES
    )

    # per slice: [lhsT-x | rhs-y | rhs-x-up | rhs-x-dg] each [38, 2, 512] f8
    XIN_d = nc.dram_tensor("XIN", (SPT, KP, 8, 512), f8, kind="ExternalInput").ap()
    ACC_d = nc.dram_tensor("ACC", (128, 2 * SPT), f32, kind="ExternalOutput").ap()

    windows, brackets_at, runs = _plan()

    with tile.TileContext(nc) as tc:
        with (
            tc.tile_pool(name="ins", bufs=3) as inpool,
            tc.tile_pool(name="ps", bufs=1, space="PSUM") as pspool,
            tc.tile_pool(name="es", bufs=8) as epool,
            tc.tile_pool(name="fix", bufs=1) as fixpool,
        ):
            ones_t = fixpool.tile([128, 1], bf16)
            nc.gpsimd.memset(ones_t[:], 1.0)
            red = pspool.tile([128, 512], f32, tag="red")
            # PE ramp warm-up (see baseline): a few tiny matmuls at t~0
            for _ in range(4):
                nc.tensor.matmul(red[0:1, 400:401], lhsT=ones_t[:, 0:1],
                                 rhs=ones_t[:], start=True, stop=True,
                                 skip_group_check=True)

            in_tiles = {}

            def get_in(s):
                if s not in in_tiles:
                    t8 = inpool.tile([KP, 8, 512], f8, tag="xin", bufs=4)
                    if s == 0:
                        # split so xy operands (sections 0:4) land first
                        nc.sync.dma_start(t8[:, 0:4], XIN_d[s][:, 0:4])
                        nc.sync.dma_start(t8[:, 4:8], XIN_d[s][:, 4:8])
                    else:
                        nc.sync.dma_start(t8[:], XIN_d[s])
                    in_tiles[s] = t8
                return in_tiles[s]

            # prefetch
            get_in(0)
            get_in(1)

            etiles = [None] * len(windows)

            def emit_bracket(s, typ):
                rl = runs.get((s, typ), [])
                # split runs into <=128-col reduce matmuls; widest first so
                # the group-opening write covers the full partition extent
                pieces = []
                for (wi, coff, width) in rl:
                    o = 0
                    while o < width:
                        take = min(128, width - o)
                        pieces.append((wi, coff + o, take))
                        o += take
                pieces.sort(key=lambda p: -p[2])
                col = 2 * s + (0 if typ == "xy" else 1)
                for pi, (wi, coff, take) in enumerate(pieces):
                    # each (s,typ) bracket is its own accumulation group in
                    # the red bank; groups are sequential in PE order.
                    nc.tensor.matmul(
                        red[0:take, col : col + 1],
                        lhsT=etiles[wi][:, coff : coff + take],
                        rhs=ones_t[:],
                        start=(pi == 0),
                        stop=(pi == len(pieces) - 1),
                    )

            for wi, w in enumerate(windows):
                for (s, typ) in brackets_at.get(wi, []):
                    emit_bracket(s, typ)
                e = w["engine"]
                used = max(seg[6] + (seg[5] - seg[4]) for seg in w["segs"])
                if e == "A":
                    win = pspool.tile([128, WA], f32, tag="wa", bufs=2)
                else:
                    win = pspool.tile([128, WD], f32, tag="wd", bufs=3)
                # prefetch input DMAs: ensure slices touched (and next) queued
                smax = max(seg[1] for seg in w["segs"])
                for ss in range(min(smax + 2, SPT)):
                    get_in(ss)
                for (kind, s, i, j, n0, n1, coff) in w["segs"]:
                    t8 = get_in(s)
                    lx = t8[:, 0:2, 128 * i : 128 * (i + 1)]
                    if kind == "xy":
                        rhs = t8[:, 2:4, n0:n1]
                    elif kind == "up":
                        rhs = t8[:, 4:6, 128 * j + n0 : 128 * j + n1]
                    else:  # dg
                        rhs = t8[:, 6:8, 128 * i + n0 : 128 * i + n1]
                    nc.tensor.matmul(
                        win[:, coff : coff + (n1 - n0)],
                        lhsT=lx,
                        rhs=rhs,
                        start=True,
                        stop=True,
                        perf_mode=DR,
                    )
                if e == "A":
                    et = epool.tile([128, WA], bf16, tag="ea", bufs=6)
                    nc.scalar.activation(et[:, 0:used], win[:, 0:used], Exp,
                                         scale=-0.5)
                else:
                    et = epool.tile([128, WD], bf16, tag="ed", bufs=10)
                    nc.vector.tensor_scalar(et[:, 0:used].bitcast(i16),
                                            win[:, 0:used], C1B, C2B, Mult, Add)
                etiles[wi] = et

            acc_sb = fixpool.tile([128, 2 * SPT], f32)
            cut = 2 * (SPT - 2)
            nc.vector.tensor_copy(acc_sb[:, 0:cut], red[:, 0:cut])
            nc.sync.dma_start(ACC_d[:, 0:cut], acc_sb[:, 0:cut])
            for (s, typ) in brackets_at.get(len(windows), []):
                emit_bracket(s, typ)
            nc.vector.tensor_copy(acc_sb[:, cut : 2 * SPT],
                                  red[:, cut : 2 * SPT])
            nc.sync.dma_start(ACC_d[:, cut : 2 * SPT],
                              acc_sb[:, cut : 2 * SPT])

    nc.compile()
    return nc


def _split_fp8(v):
    h = v.astype(F8)
    l = (v - h.astype(np.float32)).astype(F8)
    return h, l


def _terms(v, n):
    out = []
    r = np.asarray(v, np.float32)
    for _ in range(n):
        t = r.astype(F8)
        out.append(t)
        r = r - t.astype(np.float32)
    return out


def _pack_rhs(h, l, tvals):
    """rhs [38, 2, 512]: ktile0 [h;h;1,1,1;t123], ktile1 [l;l;1,1,1;t456]."""
    R = np.zeros((KP, 2, B), F8)
    tt = _terms(tvals, 6)
    R[0:16, 0] = h
    R[16:32, 0] = h
    R[0:16, 1] = l
    R[16:32, 1] = l
    R[32:35, :] = np.asarray(1.0, F8)
    for jj in range(3):
        R[35 + jj, 0] = tt[jj]
        R[35 + jj, 1] = tt[3 + jj]
    return R


def _prep_core(xs, ys, w):
    """xs, ys: (B, SPT, C) f32; w: (SPT,) -> (SPT, 38, 8, 512) f8."""
    out = np.zeros((SPT, KP, 8, B), F8)
    for s in range(SPT):
        X = xs[:, s, :].T.astype(np.float32)   # (C, B)
        Y = ys[:, s, :].T.astype(np.float32)
        hx, lx = _split_fp8(X)
        hy, ly = _split_fp8(Y)
        ax = hx.astype(np.float32) + lx.astype(np.float32)
        by = hy.astype(np.float32) + ly.astype(np.float32)
        sqx = (ax ** 2).sum(axis=0)
        sqy = (by ** 2).sum(axis=0)
        # lhsT: ktile0 [-2h;-2l;s123;1,1,1], ktile1 [-2h;-2l;s456;1,1,1]
        L = np.zeros((KP, 2, B), F8)
        n2h = (-2.0 * hx.astype(np.float32)).astype(F8)
        n2l = (-2.0 * lx.astype(np.float32)).astype(F8)
        st = _terms(sqx, 6)
        for kt in range(2):
            L[0:16, kt] = n2h
            L[16:32, kt] = n2l
            for jj in range(3):
                L[32 + jj, kt] = st[3 * kt + jj]
            L[35:38, kt] = np.asarray(1.0, F8)
        c_y = w[s] * (2.0 * (B - 1) / B)
        out[s, :, 0:2] = L
        out[s, :, 2:4] = _pack_rhs(hy, ly, sqy - 2.0 * np.log(c_y))
        out[s, :, 4:6] = _pack_rhs(hx, lx, sqx - 2.0 * np.log(2.0 * w[s]))
        out[s, :, 6:8] = _pack_rhs(hx, lx, sqx - 2.0 * np.log(w[s]))
    return out


def _run(x, y, trace=False, **kw):
    from concourse.bass_utils import run_bass_kernel_spmd

    if "nc" not in _CACHE:
        _CACHE["nc"] = _build_bass()
    nc = _CACHE["nc"]

    w = np.full(T, 2.0)
    w[0] = w[T - 1] = 1.0
    in_maps = []
    for c in range(NCORES):
        sl = slice(c * SPT, (c + 1) * SPT)
        in_maps.append({"XIN": _prep_core(x[:, sl, :], y[:, sl, :], w[sl])})

    return run_bass_kernel_spmd(
        nc, in_maps, list(range(NCORES)), trace=trace, **kw
    )


def _run_with_retries(x, y, trace=False, _trace_kw=None):
    import time as _time

    last = None
    for attempt in range(3):
        try:
            return _run(x, y, trace=trace, **(_trace_kw or {}))
        except Exception as e:  # noqa: BLE001
            last = e
            try:
                import jax

                jax.clear_caches()
                jax.clear_backends()
            except Exception:
                pass
            _time.sleep(2.0)
    import os
    import pickle
    import subprocess
    import sys
    import tempfile

    kdir = os.path.dirname(os.path.abspath(__file__))
    with tempfile.TemporaryDirectory() as td:
        inp = os.path.join(td, "io.pkl")
        with open(inp, "wb") as f:
            pickle.dump({"x": x, "y": y}, f)
        code = (
            "import pickle, sys; sys.path.insert(0, %r); import kernel as km; "
            "d = pickle.load(open(%r, 'rb')); "
            "r = km.kernel(d['x'], d['y']); "
            "pickle.dump(r, open(%r, 'wb'))"
            % (kdir, inp, inp + ".out")
        )
        for attempt in range(2):
            p = subprocess.run(
                [sys.executable, "-c", code], capture_output=True, timeout=1800
            )
            if p.returncode == 0 and os.path.exists(inp + ".out"):
                with open(inp + ".out", "rb") as f:
                    return pickle.load(f)
    raise last


def kernel(x, y, _trace=False, _trace_kw=None):
    x = np.asarray(x, np.float32)
    y = np.asarray(y, np.float32)
    res = _run_with_retries(x, y, trace=_trace, _trace_kw=_trace_kw)
    if isinstance(res, np.floating | np.ndarray):
        return res

    c_xx = 0.0
    c_xy = 0.0
    for c in range(NCORES):
        acc = np.asarray(res.results[c]["ACC"], np.float64)  # (128, 2*SPT)
        c_xy += acc[:, 0::2].sum()
        c_xx += acc[:, 1::2].sum()
    out = (c_xx - 512.0 * 254.0 - c_xy) / (B * (B - 1)) / 254.0
    if _trace:
        kernel.last_results = res
    return np.float32(out)


# revision 4
# speedup vs baseline: 1.0138x; 1.0138x over previous
"""Trainium2 Bass kernel v4 for nn_FDDiscriminator (batched RBF-Gram MMD).

Math (matches reference): see baseline docstring.  254 time-pair slices
dedup to T=128 slices with weights w in {1,2}.  Per slice t:
  Kxy (512x512, weight c_y = w*2(B-1)/B folded into exponent shift),
  Kxx upper-tri via 6 off-diag 128-blocks (c=2w) + 4 full diagonal
  128-blocks (c=w; full = 2*upper + diag, so host subtracts 512*254).

Device strategy per core (16 slices):
  ALL grams via fp8e4 DoubleRow matmuls (0.5 PE cycles/col).  Operands
  use a hi+lo fp8 split (a = h+l exact to ~2^-8) with K_eff=76 rows:
  pairs (h,h'),(l,h'),(h,l'),(l,l') give the EXACT a~.b~ plus 6-term fp8
  norm rows and 6-term shifted-norm rows -> PSUM d to ~1e-2 abs.
  exp is split between ACT (exact, 1024-col windows) and DVE
  (Schraudolph bf16: int16(d*C1+C2) bit-pattern, 512-col windows).
  Diagonal blocks are forced into ACT windows: d_mm ~ -2ln(w) + eps so
  exp -> w exactly after bf16 rounding; host subtracts 512*254.
  Reduction: per-(slice,type) accumulation brackets of bf16 matmuls
  against a ones column into one PSUM red bank (2*SPT columns),
  emitted with a 2-slice lag.  PSUM banks: ACT 2x[1024] + DVE 3x[512]
  + red = 8.  Host: out = (C_xx - 512*254 - C_xy)/(B(B-1))/254.
"""

import numpy as np
import ml_dtypes

BF16 = ml_dtypes.bfloat16
F8 = ml_dtypes.float8_e4m3

B = 512
T = 128
C = 16
NCORES = 8
SPT = T // NCORES
KP = 38          # physical contraction rows (2 k-tiles of 38 -> K_eff 76)

WA = 1024        # ACT window capacity (cols)
WD = 512         # DVE window capacity

# Schraudolph (bf16 flavour): int16(d*C1B + C2B) bit-viewed as bf16 ~ exp(-d/2)
SIGMA = 0.0575
C1B = float(np.float32(-0.5 * np.log2(np.e) * (1 << 7)))
C2B = float(np.float32((127.0 - SIGMA) * (1 << 7)))

_CACHE = {}


def _plan():
    """Static per-core schedule.

    Returns (windows, brackets_at, n_windows) where windows is a list of
      dict(engine='A'|'D', segs=[(kind, s, i, j, n0, n1, coff), ...])
    kind: 'xy' (lhsT x-strip i vs y cols n0:n1), 'up' (x-strip i vs
    x-strip j cols n0:n1), 'dg' (x-strip i vs x-strip i, c=w).
    coff = column offset in the window.  brackets_at[w] = list of
    (slice, type) brackets to emit before window w's gram matmuls.
    """
    # Per-slice segment queue: dg first (ACT-forced), then xy, then up.
    # Each seg: (kind, s, i, j, width_total) - splittable at any multiple
    # of 64 cols.
    rate = {"A": 1.0 / 1.2, "D": 1.0417}
    ovh = {"A": 235.0, "D": 128.0}
    cap = {"A": WA, "D": WD}
    t = {"A": 0.0, "D": 0.0}
    windows = []
    # window content bookkeeping for reduce: runs[(s, type)] = list of
    # (window_idx, coff, width)
    runs = {}
    # build the global segment stream
    segs = []
    for s in range(SPT):
        # slice 0: xy first so the first window only needs the first half
        # of the (split) slice-0 input DMA
        if s == 0:
            for i in range(4):
                segs.append(["xy", s, i, 0, 512])
            for i in range(4):
                segs.append(["dg", s, i, i, 128])
        else:
            for i in range(4):
                segs.append(["dg", s, i, i, 128])
            for i in range(4):
                segs.append(["xy", s, i, 0, 512])
        k = 0
        for i in range(4):
            for j in range(i + 1, 4):
                segs.append(["up", s, i, j, 128])
                k += 1
    total_cols = sum(sg[4] for sg in segs)
    done_cols = 0
    si = 0
    while si < len(segs):
        # choose engine: earliest-free; dg segments require ACT
        e = "A" if t["A"] <= t["D"] else "D"
        if segs[si][0] == "dg" and e == "D":
            e = "A"
        w = {"engine": e, "segs": []}
        coff = 0
        room = cap[e]
        # ramp-out: smaller final windows so the last exp (which gates the
        # drain chain) completes earlier
        if total_cols - done_cols < 1024:
            room = min(room, 512)
        while si < len(segs) and room > 0:
            kind, s, i, j, width = segs[si]
            if kind == "dg" and e == "D":
                break
            # never let a single matmul cross a 2KB PSUM bank boundary
            take = min(width, room, 512 - (coff % 512))
            # n0: consumed-so-far within this seg's rhs range
            full = 512 if kind == "xy" else 128
            n0 = full - width
            w["segs"].append((kind, s, i, j, n0, n0 + take, coff))
            typ = "xy" if kind == "xy" else "xx"
            runs.setdefault((s, typ), []).append((len(windows), coff, take))
            coff += take
            room -= take
            done_cols += take
            if take == width:
                si += 1
            else:
                segs[si][4] = width - take
        t[e] += ovh[e] + coff * rate[e]
        windows.append(w)

    # brackets: (s, typ) emitted before the first window of slice s+2
    first_win_of_slice = {}
    for wi, w in enumerate(windows):
        for seg in w["segs"]:
            s = seg[1]
            first_win_of_slice.setdefault(s, wi)
    brackets_at = {}
    order = []
    for s in range(SPT):
        order.append((s, "xy"))
        order.append((s, "xx"))
    for (s, typ) in order:
        at = first_win_of_slice.get(s + 2, len(windows))
        brackets_at.setdefault(at, []).append((s, typ))
    return windows, brackets_at, runs


def _build_bass():
    import concourse.bass as bass
    import concourse.bacc as bacc
    import concourse.tile as tile
    import concourse.mybir as mybir

    f32 = mybir.dt.float32
    i16 = mybir.dt.int16
    bf16 = mybir.dt.bfloat16
    f8 = mybir.dt.float8e4
    u8 = mybir.dt.uint8
    Exp = mybir.ActivationFunctionType.Exp
    Mult = mybir.AluOpType.mult
    Add = mybir.AluOpType.add
    DR = mybir.MatmulPerfMode.DoubleRow

    nc = bacc.Bacc(
        "TRN2", target_bir_lowering=False, debug=False, num_devices=NCORES
    )

    # per slice: [lhsT-x | rhs-y | rhs-x-up | rhs-x-dg] each [38, 2, 512] f8
    XIN_d = nc.dram_tensor("XIN", (SPT, KP, 8, 512), f8, kind="ExternalInput").ap()
    ACC_d = nc.dram_tensor("ACC", (128, 2 * SPT), f32, kind="ExternalOutput").ap()

    windows, brackets_at, runs = _plan()

    with tile.TileContext(nc) as tc:
        with (
            tc.tile_pool(name="ins", bufs=3) as inpool,
            tc.tile_pool(name="ps", bufs=1, space="PSUM") as pspool,
            tc.tile_pool(name="es", bufs=8) as epool,
            tc.tile_pool(name="fix", bufs=1) as fixpool,
        ):
            ones_t = fixpool.tile([128, 1], bf16)
            nc.gpsimd.memset(ones_t[:], 1.0)
            red = pspool.tile([128, 512], f32, tag="red")
            # PE ramp warm-up (see baseline): a few tiny matmuls at t~0
            for _ in range(4):
                nc.tensor.matmul(red[0:1, 400:401], lhsT=ones_t[:, 0:1],
                                 rhs=ones_t[:], start=True, stop=True,
                                 skip_group_check=True)

            in_tiles = {}

            def get_in(s):
                if s not in in_tiles:
                    t8 = inpool.tile([KP, 8, 512], f8, tag="xin", bufs=4)
                    if s == 0:
                        # split so xy operands (sections 0:4) land first
                        nc.sync.dma_start(t8[:, 0:4], XIN_d[s][:, 0:4])
                        nc.sync.dma_start(t8[:, 4:8], XIN_d[s][:, 4:8])
                    else:
                        nc.sync.dma_start(t8[:], XIN_d[s])
                    in_tiles[s] = t8
                return in_tiles[s]

            # prefetch
            get_in(0)
            get_in(1)

            etiles = [None] * len(windows)

            def emit_bracket(s, typ):
                rl = runs.get((s, typ), [])
                # split runs into <=128-col reduce matmuls; widest first so
                # the group-opening write covers the full partition extent
                pieces = []
                for (wi, coff, width) in rl:
                    o = 0
                    while o < width:
                        take = min(128, width - o)
                        pieces.append((wi, coff + o, take))
                        o += take
                pieces.sort(key=lambda p: -p[2])
                col = 2 * s + (0 if typ == "xy" else 1)
                for pi, (wi, coff, take) in enumerate(pieces):
                    # each (s,typ) bracket is its own accumulation group in
                    # the red bank; groups are sequential in PE order.
                    nc.tensor.matmul(
                        red[0:take, col : col + 1],
                        lhsT=etiles[wi][:, coff : coff + take],
                        rhs=ones_t[:],
                        start=(pi == 0),
                        stop=(pi == len(pieces) - 1),
                    )

            for wi, w in enumerate(windows):
                for (s, typ) in brackets_at.get(wi, []):
                    emit_bracket(s, typ)
                e = w["engine"]
                used = max(seg[6] + (seg[5] - seg[4]) for seg in w["segs"])
                if e == "A":
                    win = pspool.tile([128, WA], f32, tag="wa", bufs=2)
                else:
                    win = pspool.tile([128, WD], f32, tag="wd", bufs=3)
                # prefetch input DMAs: ensure slices touched (and next) queued
                smax = max(seg[1] for seg in w["segs"])
                for ss in range(min(smax + 2, SPT)):
                    get_in(ss)
                for (kind, s, i, j, n0, n1, coff) in w["segs"]:
                    t8 = get_in(s)
                    lx = t8[:, 0:2, 128 * i : 128 * (i + 1)]
                    if kind == "xy":
                        rhs = t8[:, 2:4, n0:n1]
                    elif kind == "up":
                        rhs = t8[:, 4:6, 128 * j + n0 : 128 * j + n1]
                    else:  # dg
                        rhs = t8[:, 6:8, 128 * i + n0 : 128 * i + n1]
                    nc.tensor.matmul(
                        win[:, coff : coff + (n1 - n0)],
                        lhsT=lx,
                        rhs=rhs,
                        start=True,
                        stop=True,
                        perf_mode=DR,
                    )
                if e == "A":
                    et = epool.tile([128, WA], bf16, tag="ea", bufs=6)
                    nc.scalar.activation(et[:, 0:used], win[:, 0:used], Exp,
                                         scale=-0.5)
                else:
                    et = epool.tile([128, WD], bf16, tag="ed", bufs=10)
                    nc.vector.tensor_scalar(et[:, 0:used].bitcast(i16),
                                            win[:, 0:used], C1B, C2B, Mult, Add)
                etiles[wi] = et

            acc_sb = fixpool.tile([128, 2 * SPT], f32)
            cut = 2 * (SPT - 2)
            nc.vector.tensor_copy(acc_sb[:, 0:cut], red[:, 0:cut])
            nc.sync.dma_start(ACC_d[:, 0:cut], acc_sb[:, 0:cut])
            for (s, typ) in brackets_at.get(len(windows), []):
                emit_bracket(s, typ)
            nc.vector.tensor_copy(acc_sb[:, cut : 2 * SPT],
                                  red[:, cut : 2 * SPT])
            nc.sync.dma_start(ACC_d[:, cut : 2 * SPT],
                              acc_sb[:, cut : 2 * SPT])

    nc.compile()
    return nc


def _split_fp8(v):
    h = v.astype(F8)
    l = (v - h.astype(np.float32)).astype(F8)
    return h, l


def _terms(v, n):
    out = []
    r = np.asarray(v, np.float32)
    for _ in range(n):
        t = r.astype(F8)
        out.append(t)
        r = r - t.astype(np.float32)
    return out


def _pack_rhs(h, l, tvals):
    """rhs [38, 2, 512]: ktile0 [h;h;1,1,1;t123], ktile1 [l;l;1,1,1;t456]."""
    R = np.zeros((KP, 2, B), F8)
    tt = _terms(tvals, 6)
    R[0:16, 0] = h
    R[16:32, 0] = h
    R[0:16, 1] = l
    R[16:32, 1] = l
    R[32:35, :] = np.asarray(1.0, F8)
    for jj in range(3):
        R[35 + jj, 0] = tt[jj]
        R[35 + jj, 1] = tt[3 + jj]
    return R


def _prep_core(xs, ys, w):
    """xs, ys: (B, SPT, C) f32; w: (SPT,) -> (SPT, 38, 8, 512) f8."""
    out = np.zeros((SPT, KP, 8, B), F8)
    for s in range(SPT):
        X = xs[:, s, :].T.astype(np.float32)   # (C, B)
        Y = ys[:, s, :].T.astype(np.float32)
        hx, lx = _split_fp8(X)
        hy, ly = _split_fp8(Y)
        ax = hx.astype(np.float32) + lx.astype(np.float32)
        by = hy.astype(np.float32) + ly.astype(np.float32)
        sqx = (ax ** 2).sum(axis=0)
        sqy = (by ** 2).sum(axis=0)
        # lhsT: ktile0 [-2h;-2l;s123;1,1,1], ktile1 [-2h;-2l;s456;1,1,1]
        L = np.zeros((KP, 2, B), F8)
        n2h = (-2.0 * hx.astype(np.float32)).astype(F8)
        n2l = (-2.0 * lx.astype(np.float32)).astype(F8)
        st = _terms(sqx, 6)
        for kt in range(2):
            L[0:16, kt] = n2h
            L[16:32, kt] = n2l
            for jj in range(3):
                L[32 + jj, kt] = st[3 * kt + jj]
            L[35:38, kt] = np.asarray(1.0, F8)
        c_y = w[s] * (2.0 * (B - 1) / B)
        out[s, :, 0:2] = L
        out[s, :, 2:4] = _pack_rhs(hy, ly, sqy - 2.0 * np.log(c_y))
        out[s, :, 4:6] = _pack_rhs(hx, lx, sqx - 2.0 * np.log(2.0 * w[s]))
        out[s, :, 6:8] = _pack_rhs(hx, lx, sqx - 2.0 * np.log(w[s]))
    return out


def _run(x, y, trace=False, **kw):
    from concourse.bass_utils import run_bass_kernel_spmd

    if "nc" not in _CACHE:
        _CACHE["nc"] = _build_bass()
    nc = _CACHE["nc"]

    w = np.full(T, 2.0)
    w[0] = w[T - 1] = 1.0
    in_maps = []
    for c in range(NCORES):
        sl = slice(c * SPT, (c + 1) * SPT)
        in_maps.append({"XIN": _prep_core(x[:, sl, :], y[:, sl, :], w[sl])})

    return run_bass_kernel_spmd(
        nc, in_maps, list(range(NCORES)), trace=trace, **kw
    )


def _run_with_retries(x, y, trace=False, _trace_kw=None):
    import time as _time

    last = None
    for attempt in range(3):
        try:
            return _run(x, y, trace=trace, **(_trace_kw or {}))
        except Exception as e:  # noqa: BLE001
            last = e
            try:
                import jax

                jax.clear_caches()
                jax.clear_backends()
            except Exception:
                pass
            _time.sleep(2.0)
    import os
    import pickle
    import subprocess
    import sys
    import tempfile

    kdir = os.path.dirname(os.path.abspath(__file__))
    with tempfile.TemporaryDirectory() as td:
        inp = os.path.join(td, "io.pkl")
        with open(inp, "wb") as f:
            pickle.dump({"x": x, "y": y}, f)
        code = (
            "import pickle, sys; sys.path.insert(0, %r); import kernel as km; "
            "d = pickle.load(open(%r, 'rb')); "
            "r = km.kernel(d['x'], d['y']); "
            "pickle.dump(r, open(%r, 'wb'))"
            % (kdir, inp, inp + ".out")
        )
        for attempt in range(2):
            p = subprocess.run(
                [sys.executable, "-c", code], capture_output=True, timeout=1800
            )
            if p.returncode == 0 and os.path.exists(inp + ".out"):
                with open(inp + ".out", "rb") as f:
                    return pickle.load(f)
    raise last


def kernel(x, y, _trace=False, _trace_kw=None):
    x = np.asarray(x, np.float32)
    y = np.asarray(y, np.float32)
    res = _run_with_retries(x, y, trace=_trace, _trace_kw=_trace_kw)
    if isinstance(res, np.floating | np.ndarray):
        return res

    c_xx = 0.0
    c_xy = 0.0
    for c in range(NCORES):
        acc = np.asarray(res.results[c]["ACC"], np.float64)  # (128, 2*SPT)
        c_xy += acc[:, 0::2].sum()
        c_xx += acc[:, 1::2].sum()
    out = (c_xx - 512.0 * 254.0 - c_xy) / (B * (B - 1)) / 254.0
    if _trace:
        kernel.last_results = res
    return np.float32(out)


# revision 5
# speedup vs baseline: 1.0178x; 1.0039x over previous
"""Trainium2 Bass kernel v4 for nn_FDDiscriminator (batched RBF-Gram MMD).

Math (matches reference): see baseline docstring.  254 time-pair slices
dedup to T=128 slices with weights w in {1,2}.  Per slice t:
  Kxy (512x512, weight c_y = w*2(B-1)/B folded into exponent shift),
  Kxx upper-tri via 6 off-diag 128-blocks (c=2w) + 4 full diagonal
  128-blocks (c=w; full = 2*upper + diag, so host subtracts 512*254).

Device strategy per core (16 slices):
  ALL grams via fp8e4 DoubleRow matmuls (0.5 PE cycles/col).  Operands
  use a hi+lo fp8 split (a = h+l exact to ~2^-8) with K_eff=76 rows:
  pairs (h,h'),(l,h'),(h,l'),(l,l') give the EXACT a~.b~ plus 6-term fp8
  norm rows and 6-term shifted-norm rows -> PSUM d to ~1e-2 abs.
  exp is split between ACT (exact, 1024-col windows) and DVE
  (Schraudolph bf16: int16(d*C1+C2) bit-pattern, 512-col windows).
  Diagonal blocks are forced into ACT windows: d_mm ~ -2ln(w) + eps so
  exp -> w exactly after bf16 rounding; host subtracts 512*254.
  Reduction: per-(slice,type) accumulation brackets of bf16 matmuls
  against a ones column into one PSUM red bank (2*SPT columns),
  emitted with a 2-slice lag.  PSUM banks: ACT 2x[1024] + DVE 3x[512]
  + red = 8.  Host: out = (C_xx - 512*254 - C_xy)/(B(B-1))/254.
"""

import numpy as np
import ml_dtypes

BF16 = ml_dtypes.bfloat16
F8 = ml_dtypes.float8_e4m3

B = 512
T = 128
C = 16
NCORES = 8
SPT = T // NCORES
KP = 38          # physical contraction rows (2 k-tiles of 38 -> K_eff 76)

WA = 1024        # ACT window capacity (cols)
WD = 512         # DVE window capacity

# Schraudolph (bf16 flavour): int16(d*C1B + C2B) bit-viewed as bf16 ~ exp(-d/2)
SIGMA = 0.0575
C1B = float(np.float32(-0.5 * np.log2(np.e) * (1 << 7)))
C2B = float(np.float32((127.0 - SIGMA) * (1 << 7)))

_CACHE = {}


def _plan():
    """Static per-core schedule.

    Returns (windows, brackets_at, n_windows) where windows is a list of
      dict(engine='A'|'D', segs=[(kind, s, i, j, n0, n1, coff), ...])
    kind: 'xy' (lhsT x-strip i vs y cols n0:n1), 'up' (x-strip i vs
    x-strip j cols n0:n1), 'dg' (x-strip i vs x-strip i, c=w).
    coff = column offset in the window.  brackets_at[w] = list of
    (slice, type) brackets to emit before window w's gram matmuls.
    """
    # Per-slice segment queue: dg first (ACT-forced), then xy, then up.
    # Each seg: (kind, s, i, j, width_total) - splittable at any multiple
    # of 64 cols.
    rate = {"A": 1.0 / 1.2, "D": 1.0417}
    ovh = {"A": 220.0, "D": 128.0}
    cap = {"A": WA, "D": WD}
    t = {"A": 0.0, "D": 0.0}
    windows = []
    # window content bookkeeping for reduce: runs[(s, type)] = list of
    # (window_idx, coff, width)
    runs = {}
    # build the global segment stream
    segs = []
    for s in range(SPT):
        # slice 0: xy first so the first window only needs the first half
        # of the (split) slice-0 input DMA
        if s == 0:
            for i in range(4):
                segs.append(["xy", s, i, 0, 512])
            for i in range(4):
                segs.append(["dg", s, i, i, 128])
        else:
            for i in range(4):
                segs.append(["dg", s, i, i, 128])
            for i in range(4):
                segs.append(["xy", s, i, 0, 512])
        k = 0
        for i in range(4):
            for j in range(i + 1, 4):
                segs.append(["up", s, i, j, 128])
                k += 1
    total_cols = sum(sg[4] for sg in segs)
    done_cols = 0
    si = 0
    while si < len(segs):
        # choose engine: earliest-free; dg segments require ACT
        e = "A" if t["A"] <= t["D"] else "D"
        if segs[si][0] == "dg" and e == "D":
            e = "A"
        w = {"engine": e, "segs": []}
        coff = 0
        room = cap[e]
        # ramp-out: smaller final windows so the last exp (which gates the
        # drain chain) completes earlier
        if total_cols - done_cols < 1024:
            room = min(room, 512)
        while si < len(segs) and room > 0:
            kind, s, i, j, width = segs[si]
            if kind == "dg" and e == "D":
                break
            # never let a single matmul cross a 2KB PSUM bank boundary
            take = min(width, room, 512 - (coff % 512))
            # n0: consumed-so-far within this seg's rhs range
            full = 512 if kind == "xy" else 128
            n0 = full - width
            w["segs"].append((kind, s, i, j, n0, n0 + take, coff))
            typ = "xy" if kind == "xy" else "xx"
            runs.setdefault((s, typ), []).append((len(windows), coff, take))
            coff += take
            room -= take
            done_cols += take
            if take == width:
                si += 1
            else:
                segs[si][4] = width - take
        t[e] += ovh[e] + coff * rate[e]
        windows.append(w)

    # brackets: (s, typ) emitted before the first window of slice s+2
    first_win_of_slice = {}
    for wi, w in enumerate(windows):
        for seg in w["segs"]:
            s = seg[1]
            first_win_of_slice.setdefault(s, wi)
    brackets_at = {}
    order = []
    for s in range(SPT):
        order.append((s, "xy"))
        order.append((s, "xx"))
    for (s, typ) in order:
        at = first_win_of_slice.get(s + 2, len(windows))
        brackets_at.setdefault(at, []).append((s, typ))
    return windows, brackets_at, runs


def _build_bass():
    import concourse.bass as bass
    import concourse.bacc as bacc
    import concourse.tile as tile
    import concourse.mybir as mybir

    f32 = mybir.dt.float32
    i16 = mybir.dt.int16
    bf16 = mybir.dt.bfloat16
    f8 = mybir.dt.float8e4
    u8 = mybir.dt.uint8
    Exp = mybir.ActivationFunctionType.Exp
    Mult = mybir.AluOpType.mult
    Add = mybir.AluOpType.add
    DR = mybir.MatmulPerfMode.DoubleRow

    nc = bacc.Bacc(
        "TRN2", target_bir_lowering=False, debug=False, num_devices=NCORES
    )

    # per slice: [lhsT-x | rhs-y | rhs-x-up | rhs-x-dg] each [38, 2, 512] f8
    XIN_d = nc.dram_tensor("XIN", (SPT, KP, 8, 512), f8, kind="ExternalInput").ap()
    ACC_d = nc.dram_tensor("ACC", (128, 2 * SPT), f32, kind="ExternalOutput").ap()

    windows, brackets_at, runs = _plan()

    with tile.TileContext(nc) as tc:
        with (
            tc.tile_pool(name="ins", bufs=3) as inpool,
            tc.tile_pool(name="ps", bufs=1, space="PSUM") as pspool,
            tc.tile_pool(name="es", bufs=8) as epool,
            tc.tile_pool(name="fix", bufs=1) as fixpool,
        ):
            ones_t = fixpool.tile([128, 1], bf16)
            nc.gpsimd.memset(ones_t[:], 1.0)
            red = pspool.tile([128, 512], f32, tag="red")
            # PE ramp warm-up (see baseline): a few tiny matmuls at t~0
            for _ in range(4):
                nc.tensor.matmul(red[0:1, 400:401], lhsT=ones_t[:, 0:1],
                                 rhs=ones_t[:], start=True, stop=True,
                                 skip_group_check=True)

            in_tiles = {}

            def get_in(s):
                if s not in in_tiles:
                    t8 = inpool.tile([KP, 8, 512], f8, tag="xin", bufs=4)
                    if s == 0:
                        # split so xy operands (sections 0:4) land first
                        nc.sync.dma_start(t8[:, 0:4], XIN_d[s][:, 0:4])
                        nc.sync.dma_start(t8[:, 4:8], XIN_d[s][:, 4:8])
                    else:
                        nc.sync.dma_start(t8[:], XIN_d[s])
                    in_tiles[s] = t8
                return in_tiles[s]

            # prefetch
            get_in(0)
            get_in(1)

            etiles = [None] * len(windows)

            def emit_bracket(s, typ):
                rl = runs.get((s, typ), [])
                # split runs into <=128-col reduce matmuls; widest first so
                # the group-opening write covers the full partition extent
                pieces = []
                for (wi, coff, width) in rl:
                    o = 0
                    while o < width:
                        take = min(128, width - o)
                        pieces.append((wi, coff + o, take))
                        o += take
                pieces.sort(key=lambda p: -p[2])
                col = 2 * s + (0 if typ == "xy" else 1)
                for pi, (wi, coff, take) in enumerate(pieces):
                    # each (s,typ) bracket is its own accumulation group in
                    # the red bank; groups are sequential in PE order.
                    nc.tensor.matmul(
                        red[0:take, col : col + 1],
                        lhsT=etiles[wi][:, coff : coff + take],
                        rhs=ones_t[:],
                        start=(pi == 0),
                        stop=(pi == len(pieces) - 1),
                    )

            for wi, w in enumerate(windows):
                for (s, typ) in brackets_at.get(wi, []):
                    emit_bracket(s, typ)
                e = w["engine"]
                used = max(seg[6] + (seg[5] - seg[4]) for seg in w["segs"])
                if e == "A":
                    win = pspool.tile([128, WA], f32, tag="wa", bufs=2)
                else:
                    win = pspool.tile([128, WD], f32, tag="wd", bufs=3)
                # prefetch input DMAs: ensure slices touched (and next) queued
                smax = max(seg[1] for seg in w["segs"])
                for ss in range(min(smax + 2, SPT)):
                    get_in(ss)
                for (kind, s, i, j, n0, n1, coff) in w["segs"]:
                    t8 = get_in(s)
                    lx = t8[:, 0:2, 128 * i : 128 * (i + 1)]
                    if kind == "xy":
                        rhs = t8[:, 2:4, n0:n1]
                    elif kind == "up":
                        rhs = t8[:, 4:6, 128 * j + n0 : 128 * j + n1]
                    else:  # dg
                        rhs = t8[:, 6:8, 128 * i + n0 : 128 * i + n1]
                    nc.tensor.matmul(
                        win[:, coff : coff + (n1 - n0)],
                        lhsT=lx,
                        rhs=rhs,
                        start=True,
                        stop=True,
                        perf_mode=DR,
                    )
                if e == "A":
                    et = epool.tile([128, WA], bf16, tag="ea", bufs=6)
                    nc.scalar.activation(et[:, 0:used], win[:, 0:used], Exp,
                                         scale=-0.5)
                else:
                    et = epool.tile([128, WD], bf16, tag="ed", bufs=10)
                    nc.vector.tensor_scalar(et[:, 0:used].bitcast(i16),
                                            win[:, 0:used], C1B, C2B, Mult, Add)
                etiles[wi] = et

            acc_sb = fixpool.tile([128, 2 * SPT], f32)
            cut = 2 * (SPT - 2)
            nc.vector.tensor_copy(acc_sb[:, 0:cut], red[:, 0:cut])
            nc.sync.dma_start(ACC_d[:, 0:cut], acc_sb[:, 0:cut])
            for (s, typ) in brackets_at.get(len(windows), []):
                emit_bracket(s, typ)
            nc.vector.tensor_copy(acc_sb[:, cut : 2 * SPT],
                                  red[:, cut : 2 * SPT])
            nc.sync.dma_start(ACC_d[:, cut : 2 * SPT],
                              acc_sb[:, cut : 2 * SPT])

    nc.compile()
    return nc


def _split_fp8(v):
    h = v.astype(F8)
    l = (v - h.astype(np.float32)).astype(F8)
    return h, l


def _terms(v, n):
    out = []
    r = np.asarray(v, np.float32)
    for _ in range(n):
        t = r.astype(F8)
        out.append(t)
        r = r - t.astype(np.float32)
    return out


def _pack_rhs(h, l, tvals):
    """rhs [38, 2, 512]: ktile0 [h;h;1,1,1;t123], ktile1 [l;l;1,1,1;t456]."""
    R = np.zeros((KP, 2, B), F8)
    tt = _terms(tvals, 6)
    R[0:16, 0] = h
    R[16:32, 0] = h
    R[0:16, 1] = l
    R[16:32, 1] = l
    R[32:35, :] = np.asarray(1.0, F8)
    for jj in range(3):
        R[35 + jj, 0] = tt[jj]
        R[35 + jj, 1] = tt[3 + jj]
    return R


def _prep_core(xs, ys, w):
    """xs, ys: (B, SPT, C) f32; w: (SPT,) -> (SPT, 38, 8, 512) f8."""
    out = np.zeros((SPT, KP, 8, B), F8)
    for s in range(SPT):
        X = xs[:, s, :].T.astype(np.float32)   # (C, B)
        Y = ys[:, s, :].T.astype(np.float32)
        hx, lx = _split_fp8(X)
        hy, ly = _split_fp8(Y)
        ax = hx.astype(np.float32) + lx.astype(np.float32)
        by = hy.astype(np.float32) + ly.astype(np.float32)
        sqx = (ax ** 2).sum(axis=0)
        sqy = (by ** 2).sum(axis=0)
        # lhsT: ktile0 [-2h;-2l;s123;1,1,1], ktile1 [-2h;-2l;s456;1,1,1]
        L = np.zeros((KP, 2, B), F8)
        n2h = (-2.0 * hx.astype(np.float32)).astype(F8)
        n2l = (-2.0 * lx.astype(np.float32)).astype(F8)
        st = _terms(sqx, 6)
        for kt in range(2):
            L[0:16, kt] = n2h
            L[16:32, kt] = n2l
            for jj in range(3):
                L[32 + jj, kt] = st[3 * kt + jj]
            L[35:38, kt] = np.asarray(1.0, F8)
        c_y = w[s] * (2.0 * (B - 1) / B)
        out[s, :, 0:2] = L
        out[s, :, 2:4] = _pack_rhs(hy, ly, sqy - 2.0 * np.log(c_y))
        out[s, :, 4:6] = _pack_rhs(hx, lx, sqx - 2.0 * np.log(2.0 * w[s]))
        out[s, :, 6:8] = _pack_rhs(hx, lx, sqx - 2.0 * np.log(w[s]))
    return out


def _run(x, y, trace=False, **kw):
    from concourse.bass_utils import run_bass_kernel_spmd

    if "nc" not in _CACHE:
        _CACHE["nc"] = _build_bass()
    nc = _CACHE["nc"]

    w = np.full(T, 2.0)
    w[0] = w[T - 1] = 1.0
    in_maps = []
    for c in range(NCORES):
        sl = slice(c * SPT, (c + 1) * SPT)
        in_maps.append({"XIN": _prep_core(x[:, sl, :], y[:, sl, :], w[sl])})

    return run_bass_kernel_spmd(
        nc, in_maps, list(range(NCORES)), trace=trace, **kw
    )


def _run_with_retries(x, y, trace=False, _trace_kw=None):
    import time as _time

    last = None
    for attempt in range(3):
        try:
            return _run(x, y, trace=trace, **(_trace_kw or {}))
        except Exception as e:  # noqa: BLE001
            last = e
            try:
                import jax

                jax.clear_caches()
                jax.clear_backends()
            except Exception:
                pass
            _time.sleep(2.0)
    import os
    import pickle
    import subprocess
    import sys
    import tempfile

    kdir = os.path.dirname(os.path.abspath(__file__))
    with tempfile.TemporaryDirectory() as td:
        inp = os.path.join(td, "io.pkl")
        with open(inp, "wb") as f:
            pickle.dump({"x": x, "y": y}, f)
        code = (
            "import pickle, sys; sys.path.insert(0, %r); import kernel as km; "
            "d = pickle.load(open(%r, 'rb')); "
            "r = km.kernel(d['x'], d['y']); "
            "pickle.dump(r, open(%r, 'wb'))"
            % (kdir, inp, inp + ".out")
        )
        for attempt in range(2):
            p = subprocess.run(
                [sys.executable, "-c", code], capture_output=True, timeout=1800
            )
            if p.returncode == 0 and os.path.exists(inp + ".out"):
                with open(inp + ".out", "rb") as f:
                    return pickle.load(f)
    raise last


def kernel(x, y, _trace=False, _trace_kw=None):
    x = np.asarray(x, np.float32)
    y = np.asarray(y, np.float32)
    res = _run_with_retries(x, y, trace=_trace, _trace_kw=_trace_kw)
    if isinstance(res, np.floating | np.ndarray):
        return res

    c_xx = 0.0
    c_xy = 0.0
    for c in range(NCORES):
        acc = np.asarray(res.results[c]["ACC"], np.float64)  # (128, 2*SPT)
        c_xy += acc[:, 0::2].sum()
        c_xx += acc[:, 1::2].sum()
    out = (c_xx - 512.0 * 254.0 - c_xy) / (B * (B - 1)) / 254.0
    if _trace:
        kernel.last_results = res
    return np.float32(out)


# revision 6
# speedup vs baseline: 1.0179x; 1.0001x over previous
"""Trainium2 Bass kernel v4 for nn_FDDiscriminator (batched RBF-Gram MMD).

Math (matches reference): see baseline docstring.  254 time-pair slices
dedup to T=128 slices with weights w in {1,2}.  Per slice t:
  Kxy (512x512, weight c_y = w*2(B-1)/B folded into exponent shift),
  Kxx upper-tri via 6 off-diag 128-blocks (c=2w) + 4 full diagonal
  128-blocks (c=w; full = 2*upper + diag, so host subtracts 512*254).

Device strategy per core (16 slices):
  ALL grams via fp8e4 DoubleRow matmuls (0.5 PE cycles/col).  Operands
  use a hi+lo fp8 split (a = h+l exact to ~2^-8) with K_eff=76 rows:
  pairs (h,h'),(l,h'),(h,l'),(l,l') give the EXACT a~.b~ plus 6-term fp8
  norm rows and 6-term shifted-norm rows -> PSUM d to ~1e-2 abs.
  exp is split between ACT (exact, 1024-col windows) and DVE
  (Schraudolph bf16: int16(d*C1+C2) bit-pattern, 512-col windows).
  Diagonal blocks are forced into ACT windows: d_mm ~ -2ln(w) + eps so
  exp -> w exactly after bf16 rounding; host subtracts 512*254.
  Reduction: per-(slice,type) accumulation brackets of bf16 matmuls
  against a ones column into one PSUM red bank (2*SPT columns),
  emitted with a 2-slice lag.  PSUM banks: ACT 2x[1024] + DVE 3x[512]
  + red = 8.  Host: out = (C_xx - 512*254 - C_xy)/(B(B-1))/254.
"""

import numpy as np
import ml_dtypes

BF16 = ml_dtypes.bfloat16
F8 = ml_dtypes.float8_e4m3

B = 512
T = 128
C = 16
NCORES = 8
SPT = T // NCORES
KP = 38          # physical contraction rows (2 k-tiles of 38 -> K_eff 76)

WA = 1024        # ACT window capacity (cols)
WD = 512         # DVE window capacity

# Schraudolph (bf16 flavour): int16(d*C1B + C2B) bit-viewed as bf16 ~ exp(-d/2)
SIGMA = 0.0575
C1B = float(np.float32(-0.5 * np.log2(np.e) * (1 << 7)))
C2B = float(np.float32((127.0 - SIGMA) * (1 << 7)))

_CACHE = {}


def _plan():
    """Static per-core schedule.

    Returns (windows, brackets_at, n_windows) where windows is a list of
      dict(engine='A'|'D', segs=[(kind, s, i, j, n0, n1, coff), ...])
    kind: 'xy' (lhsT x-strip i vs y cols n0:n1), 'up' (x-strip i vs
    x-strip j cols n0:n1), 'dg' (x-strip i vs x-strip i, c=w).
    coff = column offset in the window.  brackets_at[w] = list of
    (slice, type) brackets to emit before window w's gram matmuls.
    """
    # Per-slice segment queue: dg first (ACT-forced), then xy, then up.
    # Each seg: (kind, s, i, j, width_total) - splittable at any multiple
    # of 64 cols.
    rate = {"A": 1.0 / 1.2, "D": 1.0417}
    ovh = {"A": 220.0, "D": 128.0}
    cap = {"A": WA, "D": WD}
    t = {"A": 0.0, "D": 0.0}
    windows = []
    # window content bookkeeping for reduce: runs[(s, type)] = list of
    # (window_idx, coff, width)
    runs = {}
    # build the global segment stream
    segs = []
    for s in range(SPT):
        # slice 0: xy first so the first window only needs the first half
        # of the (split) slice-0 input DMA
        if s == 0:
            for i in range(4):
                segs.append(["xy", s, i, 0, 512])
            for i in range(4):
                segs.append(["dg", s, i, i, 128])
        else:
            for i in range(4):
                segs.append(["dg", s, i, i, 128])
            for i in range(4):
                segs.append(["xy", s, i, 0, 512])
        k = 0
        for i in range(4):
            for j in range(i + 1, 4):
                segs.append(["up", s, i, j, 128])
                k += 1
    total_cols = sum(sg[4] for sg in segs)
    done_cols = 0
    si = 0
    while si < len(segs):
        # choose engine: earliest-free; dg segments require ACT
        e = "A" if t["A"] <= t["D"] else "D"
        if segs[si][0] == "dg" and e == "D":
            e = "A"
        w = {"engine": e, "segs": []}
        coff = 0
        room = cap[e]
        # ramp-out: smaller final windows so the last exp (which gates the
        # drain chain) completes earlier
        if total_cols - done_cols < 1024:
            room = min(room, 512)
        while si < len(segs) and room > 0:
            kind, s, i, j, width = segs[si]
            if kind == "dg" and e == "D":
                break
            # never let a single matmul cross a 2KB PSUM bank boundary
            take = min(width, room, 512 - (coff % 512))
            # n0: consumed-so-far within this seg's rhs range
            full = 512 if kind == "xy" else 128
            n0 = full - width
            w["segs"].append((kind, s, i, j, n0, n0 + take, coff))
            typ = "xy" if kind == "xy" else "xx"
            runs.setdefault((s, typ), []).append((len(windows), coff, take))
            coff += take
            room -= take
            done_cols += take
            if take == width:
                si += 1
            else:
                segs[si][4] = width - take
        t[e] += ovh[e] + coff * rate[e]
        windows.append(w)

    # brackets: (s, typ) emitted before the first window of slice s+2
    first_win_of_slice = {}
    for wi, w in enumerate(windows):
        for seg in w["segs"]:
            s = seg[1]
            first_win_of_slice.setdefault(s, wi)
    brackets_at = {}
    order = []
    for s in range(SPT):
        order.append((s, "xy"))
        order.append((s, "xx"))
    for (s, typ) in order:
        at = first_win_of_slice.get(s + 2, len(windows))
        brackets_at.setdefault(at, []).append((s, typ))
    return windows, brackets_at, runs


def _build_bass():
    import concourse.bass as bass
    import concourse.bacc as bacc
    import concourse.tile as tile
    import concourse.mybir as mybir

    f32 = mybir.dt.float32
    i16 = mybir.dt.int16
    bf16 = mybir.dt.bfloat16
    f8 = mybir.dt.float8e4
    u8 = mybir.dt.uint8
    Exp = mybir.ActivationFunctionType.Exp
    Mult = mybir.AluOpType.mult
    Add = mybir.AluOpType.add
    DR = mybir.MatmulPerfMode.DoubleRow

    nc = bacc.Bacc(
        "TRN2", target_bir_lowering=False, debug=False, num_devices=NCORES
    )

    # per slice: [lhsT-x | rhs-y | rhs-x-up | rhs-x-dg] each [38, 2, 512] f8
    XIN_d = nc.dram_tensor("XIN", (SPT, KP, 8, 512), f8, kind="ExternalInput").ap()
    ACC_d = nc.dram_tensor("ACC", (128, 2 * SPT), f32, kind="ExternalOutput").ap()

    windows, brackets_at, runs = _plan()

    with tile.TileContext(nc) as tc:
        with (
            tc.tile_pool(name="ins", bufs=3) as inpool,
            tc.tile_pool(name="ps", bufs=1, space="PSUM") as pspool,
            tc.tile_pool(name="es", bufs=8) as epool,
            tc.tile_pool(name="fix", bufs=1) as fixpool,
        ):
            ones_t = fixpool.tile([128, 1], bf16)
            nc.gpsimd.memset(ones_t[:], 1.0)
            red = pspool.tile([128, 512], f32, tag="red")
            # PE ramp warm-up (see baseline): a few tiny matmuls at t~0
            for _ in range(4):
                nc.tensor.matmul(red[0:1, 400:401], lhsT=ones_t[:, 0:1],
                                 rhs=ones_t[:], start=True, stop=True,
                                 skip_group_check=True)

            in_tiles = {}

            def get_in(s):
                if s not in in_tiles:
                    t8 = inpool.tile([KP, 8, 512], f8, tag="xin", bufs=4)
                    if s == 0:
                        # split so xy operands (sections 0:4) land first
                        nc.sync.dma_start(t8[:, 0:4], XIN_d[s][:, 0:4])
                        nc.sync.dma_start(t8[:, 4:8], XIN_d[s][:, 4:8])
                    else:
                        nc.sync.dma_start(t8[:], XIN_d[s])
                    in_tiles[s] = t8
                return in_tiles[s]

            # prefetch
            get_in(0)
            get_in(1)

            etiles = [None] * len(windows)

            def emit_bracket(s, typ):
                rl = runs.get((s, typ), [])
                # split runs into <=128-col reduce matmuls; widest first so
                # the group-opening write covers the full partition extent
                pieces = []
                for (wi, coff, width) in rl:
                    o = 0
                    while o < width:
                        take = min(128, width - o)
                        pieces.append((wi, coff + o, take))
                        o += take
                pieces.sort(key=lambda p: -p[2])
                col = 2 * s + (0 if typ == "xy" else 1)
                for pi, (wi, coff, take) in enumerate(pieces):
                    # each (s,typ) bracket is its own accumulation group in
                    # the red bank; groups are sequential in PE order.
                    nc.tensor.matmul(
                        red[0:take, col : col + 1],
                        lhsT=etiles[wi][:, coff : coff + take],
                        rhs=ones_t[:],
                        start=(pi == 0),
                        stop=(pi == len(pieces) - 1),
                    )

            for wi, w in enumerate(windows):
                for (s, typ) in brackets_at.get(wi, []):
                    emit_bracket(s, typ)
                e = w["engine"]
                used = max(seg[6] + (seg[5] - seg[4]) for seg in w["segs"])
                if e == "A":
                    win = pspool.tile([128, WA], f32, tag="wa", bufs=2)
                else:
                    win = pspool.tile([128, WD], f32, tag="wd", bufs=3)
                # prefetch input DMAs: ensure slices touched (and next) queued
                smax = max(seg[1] for seg in w["segs"])
                for ss in range(min(smax + 2, SPT)):
                    get_in(ss)
                for (kind, s, i, j, n0, n1, coff) in w["segs"]:
                    t8 = get_in(s)
                    lx = t8[:, 0:2, 128 * i : 128 * (i + 1)]
                    if kind == "xy":
                        rhs = t8[:, 2:4, n0:n1]
                    elif kind == "up":
                        rhs = t8[:, 4:6, 128 * j + n0 : 128 * j + n1]
                    else:  # dg
                        rhs = t8[:, 6:8, 128 * i + n0 : 128 * i + n1]
                    nc.tensor.matmul(
                        win[:, coff : coff + (n1 - n0)],
                        lhsT=lx,
                        rhs=rhs,
                        start=True,
                        stop=True,
                        perf_mode=DR,
                    )
                if e == "A":
                    et = epool.tile([128, WA], bf16, tag="ea", bufs=5)
                    nc.scalar.activation(et[:, 0:used], win[:, 0:used], Exp,
                                         scale=-0.5)
                else:
                    et = epool.tile([128, WD], bf16, tag="ed", bufs=8)
                    nc.vector.tensor_scalar(et[:, 0:used].bitcast(i16),
                                            win[:, 0:used], C1B, C2B, Mult, Add)
                etiles[wi] = et

            acc_sb = fixpool.tile([128, 2 * SPT], f32)
            cut = 2 * (SPT - 2)
            nc.vector.tensor_copy(acc_sb[:, 0:cut], red[:, 0:cut])
            nc.sync.dma_start(ACC_d[:, 0:cut], acc_sb[:, 0:cut])
            for (s, typ) in brackets_at.get(len(windows), []):
                emit_bracket(s, typ)
            nc.vector.tensor_copy(acc_sb[:, cut : 2 * SPT],
                                  red[:, cut : 2 * SPT])
            nc.sync.dma_start(ACC_d[:, cut : 2 * SPT],
                              acc_sb[:, cut : 2 * SPT])

    nc.compile()
    return nc


def _split_fp8(v):
    h = v.astype(F8)
    l = (v - h.astype(np.float32)).astype(F8)
    return h, l


def _terms(v, n):
    out = []
    r = np.asarray(v, np.float32)
    for _ in range(n):
        t = r.astype(F8)
        out.append(t)
        r = r - t.astype(np.float32)
    return out


def _pack_rhs(h, l, tvals):
    """rhs [38, 2, 512]: ktile0 [h;h;1,1,1;t123], ktile1 [l;l;1,1,1;t456]."""
    R = np.zeros((KP, 2, B), F8)
    tt = _terms(tvals, 6)
    R[0:16, 0] = h
    R[16:32, 0] = h
    R[0:16, 1] = l
    R[16:32, 1] = l
    R[32:35, :] = np.asarray(1.0, F8)
    for jj in range(3):
        R[35 + jj, 0] = tt[jj]
        R[35 + jj, 1] = tt[3 + jj]
    return R


def _prep_core(xs, ys, w):
    """xs, ys: (B, SPT, C) f32; w: (SPT,) -> (SPT, 38, 8, 512) f8."""
    out = np.zeros((SPT, KP, 8, B), F8)
    for s in range(SPT):
        X = xs[:, s, :].T.astype(np.float32)   # (C, B)
        Y = ys[:, s, :].T.astype(np.float32)
        hx, lx = _split_fp8(X)
        hy, ly = _split_fp8(Y)
        ax = hx.astype(np.float32) + lx.astype(np.float32)
        by = hy.astype(np.float32) + ly.astype(np.float32)
        sqx = (ax ** 2).sum(axis=0)
        sqy = (by ** 2).sum(axis=0)
        # lhsT: ktile0 [-2h;-2l;s123;1,1,1], ktile1 [-2h;-2l;s456;1,1,1]
        L = np.zeros((KP, 2, B), F8)
        n2h = (-2.0 * hx.astype(np.float32)).astype(F8)
        n2l = (-2.0 * lx.astype(np.float32)).astype(F8)
        st = _terms(sqx, 6)
        for kt in range(2):
            L[0:16, kt] = n2h
            L[16:32, kt] = n2l
            for jj in range(3):
                L[32 + jj, kt] = st[3 * kt + jj]
            L[35:38, kt] = np.asarray(1.0, F8)
        c_y = w[s] * (2.0 * (B - 1) / B)
        out[s, :, 0:2] = L
        out[s, :, 2:4] = _pack_rhs(hy, ly, sqy - 2.0 * np.log(c_y))
        out[s, :, 4:6] = _pack_rhs(hx, lx, sqx - 2.0 * np.log(2.0 * w[s]))
        out[s, :, 6:8] = _pack_rhs(hx, lx, sqx - 2.0 * np.log(w[s]))
    return out


def _run(x, y, trace=False, **kw):
    from concourse.bass_utils import run_bass_kernel_spmd

    if "nc" not in _CACHE:
        _CACHE["nc"] = _build_bass()
    nc = _CACHE["nc"]

    w = np.full(T, 2.0)
    w[0] = w[T - 1] = 1.0
    in_maps = []
    for c in range(NCORES):
        sl = slice(c * SPT, (c + 1) * SPT)
        in_maps.append({"XIN": _prep_core(x[:, sl, :], y[:, sl, :], w[sl])})

    return run_bass_kernel_spmd(
        nc, in_maps, list(range(NCORES)), trace=trace, **kw
    )


def _run_with_retries(x, y, trace=False, _trace_kw=None):
    import time as _time

    last = None
    for attempt in range(3):
        try:
            return _run(x, y, trace=trace, **(_trace_kw or {}))
        except Exception as e:  # noqa: BLE001
            last = e
            try:
                import jax

                jax.clear_caches()
                jax.clear_backends()
            except Exception:
                pass
            _time.sleep(2.0)
    import os
    import pickle
    import subprocess
    import sys
    import tempfile

    kdir = os.path.dirname(os.path.abspath(__file__))
    with tempfile.TemporaryDirectory() as td:
        inp = os.path.join(td, "io.pkl")
        with open(inp, "wb") as f:
            pickle.dump({"x": x, "y": y}, f)
        code = (
            "import pickle, sys; sys.path.insert(0, %r); import kernel as km; "
            "d = pickle.load(open(%r, 'rb')); "
            "r = km.kernel(d['x'], d['y']); "
            "pickle.dump(r, open(%r, 'wb'))"
            % (kdir, inp, inp + ".out")
        )
        for attempt in range(2):
            p = subprocess.run(
                [sys.executable, "-c", code], capture_output=True, timeout=1800
            )
            if p.returncode == 0 and os.path.exists(inp + ".out"):
                with open(inp + ".out", "rb") as f:
                    return pickle.load(f)
    raise last


def kernel(x, y, _trace=False, _trace_kw=None):
    x = np.asarray(x, np.float32)
    y = np.asarray(y, np.float32)
    res = _run_with_retries(x, y, trace=_trace, _trace_kw=_trace_kw)
    if isinstance(res, np.floating | np.ndarray):
        return res

    c_xx = 0.0
    c_xy = 0.0
    for c in range(NCORES):
        acc = np.asarray(res.results[c]["ACC"], np.float64)  # (128, 2*SPT)
        c_xy += acc[:, 0::2].sum()
        c_xx += acc[:, 1::2].sum()
    out = (c_xx - 512.0 * 254.0 - c_xy) / (B * (B - 1)) / 254.0
    if _trace:
        kernel.last_results = res
    return np.float32(out)


# revision 7
# speedup vs baseline: 1.0205x; 1.0025x over previous
"""Trainium2 Bass kernel v4 for nn_FDDiscriminator (batched RBF-Gram MMD).

Math (matches reference): see baseline docstring.  254 time-pair slices
dedup to T=128 slices with weights w in {1,2}.  Per slice t:
  Kxy (512x512, weight c_y = w*2(B-1)/B folded into exponent shift),
  Kxx upper-tri via 6 off-diag 128-blocks (c=2w) + 4 full diagonal
  128-blocks (c=w; full = 2*upper + diag, so host subtracts 512*254).

Device strategy per core (16 slices):
  ALL grams via fp8e4 DoubleRow matmuls (0.5 PE cycles/col).  Operands
  use a hi+lo fp8 split (a = h+l exact to ~2^-8) with K_eff=76 rows:
  pairs (h,h'),(l,h'),(h,l'),(l,l') give the EXACT a~.b~ plus 6-term fp8
  norm rows and 6-term shifted-norm rows -> PSUM d to ~1e-2 abs.
  exp is split between ACT (exact, 1024-col windows) and DVE
  (Schraudolph bf16: int16(d*C1+C2) bit-pattern, 512-col windows).
  Diagonal blocks are forced into ACT windows: d_mm ~ -2ln(w) + eps so
  exp -> w exactly after bf16 rounding; host subtracts 512*254.
  Reduction: per-(slice,type) accumulation brackets of bf16 matmuls
  against a ones column into one PSUM red bank (2*SPT columns),
  emitted with a 2-slice lag.  PSUM banks: ACT 2x[1024] + DVE 3x[512]
  + red = 8.  Host: out = (C_xx - 512*254 - C_xy)/(B(B-1))/254.
"""

import numpy as np
import ml_dtypes

BF16 = ml_dtypes.bfloat16
F8 = ml_dtypes.float8_e4m3

B = 512
T = 128
C = 16
NCORES = 8
SPT = T // NCORES
KP = 38          # physical contraction rows (2 k-tiles of 38 -> K_eff 76)

WA = 1024        # ACT window capacity (cols)
WD = 512         # DVE window capacity

# Schraudolph (bf16 flavour): int16(d*C1B + C2B) bit-viewed as bf16 ~ exp(-d/2)
SIGMA = 0.0575
C1B = float(np.float32(-0.5 * np.log2(np.e) * (1 << 7)))
C2B = float(np.float32((127.0 - SIGMA) * (1 << 7)))

_CACHE = {}


def _plan():
    """Static per-core schedule.

    Returns (windows, brackets_at, n_windows) where windows is a list of
      dict(engine='A'|'D', segs=[(kind, s, i, j, n0, n1, coff), ...])
    kind: 'xy' (lhsT x-strip i vs y cols n0:n1), 'up' (x-strip i vs
    x-strip j cols n0:n1), 'dg' (x-strip i vs x-strip i, c=w).
    coff = column offset in the window.  brackets_at[w] = list of
    (slice, type) brackets to emit before window w's gram matmuls.
    """
    # Per-slice segment queue: dg first (ACT-forced), then xy, then up.
    # Each seg: (kind, s, i, j, width_total) - splittable at any multiple
    # of 64 cols.
    rate = {"A": 1.0 / 1.2, "D": 1.0417}
    ovh = {"A": 220.0, "D": 128.0}
    cap = {"A": WA, "D": WD}
    t = {"A": 0.0, "D": 0.0}
    windows = []
    # window content bookkeeping for reduce: runs[(s, type)] = list of
    # (window_idx, coff, width)
    runs = {}
    # build the global segment stream
    segs = []
    for s in range(SPT):
        # slice 0: xy first so the first window only needs the first half
        # of the (split) slice-0 input DMA
        if s == 0:
            for i in range(4):
                segs.append(["xy", s, i, 0, 512])
            for i in range(4):
                segs.append(["dg", s, i, i, 128])
        else:
            for i in range(4):
                segs.append(["dg", s, i, i, 128])
            for i in range(4):
                segs.append(["xy", s, i, 0, 512])
        k = 0
        for i in range(4):
            for j in range(i + 1, 4):
                segs.append(["up", s, i, j, 128])
                k += 1
    total_cols = sum(sg[4] for sg in segs)
    done_cols = 0
    si = 0
    while si < len(segs):
        # choose engine: earliest-free; dg segments require ACT
        e = "A" if t["A"] <= t["D"] else "D"
        if segs[si][0] == "dg" and e == "D":
            e = "A"
        w = {"engine": e, "segs": []}
        coff = 0
        room = cap[e]
        # ramp-out: smaller final windows so the last exp (which gates the
        # drain chain) completes earlier
        if total_cols - done_cols < 1024:
            room = min(room, 512)
        while si < len(segs) and room > 0:
            kind, s, i, j, width = segs[si]
            if kind == "dg" and e == "D":
                break
            # never let a single matmul cross a 2KB PSUM bank boundary
            take = min(width, room, 512 - (coff % 512))
            # n0: consumed-so-far within this seg's rhs range
            full = 512 if kind == "xy" else 128
            n0 = full - width
            w["segs"].append((kind, s, i, j, n0, n0 + take, coff))
            typ = "xy" if kind == "xy" else "xx"
            runs.setdefault((s, typ), []).append((len(windows), coff, take))
            coff += take
            room -= take
            done_cols += take
            if take == width:
                si += 1
            else:
                segs[si][4] = width - take
        t[e] += ovh[e] + coff * rate[e]
        windows.append(w)

    # brackets: (s, typ) emitted before the first window of slice s+2
    first_win_of_slice = {}
    for wi, w in enumerate(windows):
        for seg in w["segs"]:
            s = seg[1]
            first_win_of_slice.setdefault(s, wi)
    brackets_at = {}
    order = []
    for s in range(SPT):
        order.append((s, "xy"))
        order.append((s, "xx"))
    for (s, typ) in order:
        at = first_win_of_slice.get(s + 2, len(windows))
        brackets_at.setdefault(at, []).append((s, typ))
    return windows, brackets_at, runs


def _build_bass():
    import concourse.bass as bass
    import concourse.bacc as bacc
    import concourse.tile as tile
    import concourse.mybir as mybir

    f32 = mybir.dt.float32
    i16 = mybir.dt.int16
    bf16 = mybir.dt.bfloat16
    f8 = mybir.dt.float8e4
    u8 = mybir.dt.uint8
    Exp = mybir.ActivationFunctionType.Exp
    Mult = mybir.AluOpType.mult
    Add = mybir.AluOpType.add
    DR = mybir.MatmulPerfMode.DoubleRow

    nc = bacc.Bacc(
        "TRN2", target_bir_lowering=False, debug=False, num_devices=NCORES
    )

    # per slice: [lhsT-x | rhs-y | rhs-x-up | rhs-x-dg] each [38, 2, 512] f8
    XIN_d = nc.dram_tensor("XIN", (SPT, KP, 8, 512), f8, kind="ExternalInput").ap()
    ACC_d = nc.dram_tensor("ACC", (128, 2 * SPT), f32, kind="ExternalOutput").ap()

    windows, brackets_at, runs = _plan()

    with tile.TileContext(nc) as tc:
        with (
            tc.tile_pool(name="ins", bufs=3) as inpool,
            tc.tile_pool(name="ps", bufs=1, space="PSUM") as pspool,
            tc.tile_pool(name="es", bufs=8) as epool,
            tc.tile_pool(name="fix", bufs=1) as fixpool,
        ):
            ones_t = fixpool.tile([128, 1], bf16)
            nc.gpsimd.memset(ones_t[:], 1.0)
            red = pspool.tile([128, 512], f32, tag="red")
            # PE ramp warm-up (see baseline): a few tiny matmuls at t~0
            for _ in range(4):
                nc.tensor.matmul(red[0:1, 400:401], lhsT=ones_t[:, 0:1],
                                 rhs=ones_t[:], start=True, stop=True,
                                 skip_group_check=True)

            in_tiles = {}

            def get_in(s):
                if s not in in_tiles:
                    t8 = inpool.tile([KP, 8, 512], f8, tag="xin", bufs=3)
                    if s == 0:
                        # split so xy operands (sections 0:4) land first
                        nc.sync.dma_start(t8[:, 0:4], XIN_d[s][:, 0:4])
                        nc.sync.dma_start(t8[:, 4:8], XIN_d[s][:, 4:8])
                    else:
                        nc.sync.dma_start(t8[:], XIN_d[s])
                    in_tiles[s] = t8
                return in_tiles[s]

            # prefetch
            get_in(0)
            get_in(1)

            etiles = [None] * len(windows)

            def emit_bracket(s, typ):
                rl = runs.get((s, typ), [])
                # split runs into <=128-col reduce matmuls; widest first so
                # the group-opening write covers the full partition extent
                pieces = []
                for (wi, coff, width) in rl:
                    o = 0
                    while o < width:
                        take = min(128, width - o)
                        pieces.append((wi, coff + o, take))
                        o += take
                pieces.sort(key=lambda p: -p[2])
                col = 2 * s + (0 if typ == "xy" else 1)
                for pi, (wi, coff, take) in enumerate(pieces):
                    # each (s,typ) bracket is its own accumulation group in
                    # the red bank; groups are sequential in PE order.
                    nc.tensor.matmul(
                        red[0:take, col : col + 1],
                        lhsT=etiles[wi][:, coff : coff + take],
                        rhs=ones_t[:],
                        start=(pi == 0),
                        stop=(pi == len(pieces) - 1),
                    )

            for wi, w in enumerate(windows):
                for (s, typ) in brackets_at.get(wi, []):
                    emit_bracket(s, typ)
                e = w["engine"]
                used = max(seg[6] + (seg[5] - seg[4]) for seg in w["segs"])
                if e == "A":
                    win = pspool.tile([128, WA], f32, tag="wa", bufs=2)
                else:
                    win = pspool.tile([128, WD], f32, tag="wd", bufs=3)
                # prefetch input DMAs: ensure slices touched (and next) queued
                smax = max(seg[1] for seg in w["segs"])
                for ss in range(min(smax + 2, SPT)):
                    get_in(ss)
                for (kind, s, i, j, n0, n1, coff) in w["segs"]:
                    t8 = get_in(s)
                    lx = t8[:, 0:2, 128 * i : 128 * (i + 1)]
                    if kind == "xy":
                        rhs = t8[:, 2:4, n0:n1]
                    elif kind == "up":
                        rhs = t8[:, 4:6, 128 * j + n0 : 128 * j + n1]
                    else:  # dg
                        rhs = t8[:, 6:8, 128 * i + n0 : 128 * i + n1]
                    nc.tensor.matmul(
                        win[:, coff : coff + (n1 - n0)],
                        lhsT=lx,
                        rhs=rhs,
                        start=True,
                        stop=True,
                        perf_mode=DR,
                    )
                if e == "A":
                    et = epool.tile([128, WA], bf16, tag="ea", bufs=5)
                    nc.scalar.activation(et[:, 0:used], win[:, 0:used], Exp,
                                         scale=-0.5)
                else:
                    et = epool.tile([128, WD], bf16, tag="ed", bufs=8)
                    nc.vector.tensor_scalar(et[:, 0:used].bitcast(i16),
                                            win[:, 0:used], C1B, C2B, Mult, Add)
                etiles[wi] = et

            acc_sb = fixpool.tile([128, 2 * SPT], f32)
            cut = 2 * (SPT - 2)
            nc.vector.tensor_copy(acc_sb[:, 0:cut], red[:, 0:cut])
            nc.sync.dma_start(ACC_d[:, 0:cut], acc_sb[:, 0:cut])
            for (s, typ) in brackets_at.get(len(windows), []):
                emit_bracket(s, typ)
            nc.vector.tensor_copy(acc_sb[:, cut : 2 * SPT],
                                  red[:, cut : 2 * SPT])
            nc.sync.dma_start(ACC_d[:, cut : 2 * SPT],
                              acc_sb[:, cut : 2 * SPT])

    nc.compile()
    return nc


def _split_fp8(v):
    h = v.astype(F8)
    l = (v - h.astype(np.float32)).astype(F8)
    return h, l


def _terms(v, n):
    out = []
    r = np.asarray(v, np.float32)
    for _ in range(n):
        t = r.astype(F8)
        out.append(t)
        r = r - t.astype(np.float32)
    return out


def _pack_rhs(h, l, tvals):
    """rhs [38, 2, 512]: ktile0 [h;h;1,1,1;t123], ktile1 [l;l;1,1,1;t456]."""
    R = np.zeros((KP, 2, B), F8)
    tt = _terms(tvals, 6)
    R[0:16, 0] = h
    R[16:32, 0] = h
    R[0:16, 1] = l
    R[16:32, 1] = l
    R[32:35, :] = np.asarray(1.0, F8)
    for jj in range(3):
        R[35 + jj, 0] = tt[jj]
        R[35 + jj, 1] = tt[3 + jj]
    return R


def _prep_core(xs, ys, w):
    """xs, ys: (B, SPT, C) f32; w: (SPT,) -> (SPT, 38, 8, 512) f8."""
    out = np.zeros((SPT, KP, 8, B), F8)
    for s in range(SPT):
        X = xs[:, s, :].T.astype(np.float32)   # (C, B)
        Y = ys[:, s, :].T.astype(np.float32)
        hx, lx = _split_fp8(X)
        hy, ly = _split_fp8(Y)
        ax = hx.astype(np.float32) + lx.astype(np.float32)
        by = hy.astype(np.float32) + ly.astype(np.float32)
        sqx = (ax ** 2).sum(axis=0)
        sqy = (by ** 2).sum(axis=0)
        # lhsT: ktile0 [-2h;-2l;s123;1,1,1], ktile1 [-2h;-2l;s456;1,1,1]
        L = np.zeros((KP, 2, B), F8)
        n2h = (-2.0 * hx.astype(np.float32)).astype(F8)
        n2l = (-2.0 * lx.astype(np.float32)).astype(F8)
        st = _terms(sqx, 6)
        for kt in range(2):
            L[0:16, kt] = n2h
            L[16:32, kt] = n2l
            for jj in range(3):
                L[32 + jj, kt] = st[3 * kt + jj]
            L[35:38, kt] = np.asarray(1.0, F8)
        c_y = w[s] * (2.0 * (B - 1) / B)
        out[s, :, 0:2] = L
        out[s, :, 2:4] = _pack_rhs(hy, ly, sqy - 2.0 * np.log(c_y))
        out[s, :, 4:6] = _pack_rhs(hx, lx, sqx - 2.0 * np.log(2.0 * w[s]))
        out[s, :, 6:8] = _pack_rhs(hx, lx, sqx - 2.0 * np.log(w[s]))
    return out


def _run(x, y, trace=False, **kw):
    from concourse.bass_utils import run_bass_kernel_spmd

    if "nc" not in _CACHE:
        _CACHE["nc"] = _build_bass()
    nc = _CACHE["nc"]

    w = np.full(T, 2.0)
    w[0] = w[T - 1] = 1.0
    in_maps = []
    for c in range(NCORES):
        sl = slice(c * SPT, (c + 1) * SPT)
        in_maps.append({"XIN": _prep_core(x[:, sl, :], y[:, sl, :], w[sl])})

    return run_bass_kernel_spmd(
        nc, in_maps, list(range(NCORES)), trace=trace, **kw
    )


def _run_with_retries(x, y, trace=False, _trace_kw=None):
    import time as _time

    last = None
    for attempt in range(3):
        try:
            return _run(x, y, trace=trace, **(_trace_kw or {}))
        except Exception as e:  # noqa: BLE001
            last = e
            try:
                import jax

                jax.clear_caches()
                jax.clear_backends()
            except Exception:
                pass
            _time.sleep(2.0)
    import os
    import pickle
    import subprocess
    import sys
    import tempfile

    kdir = os.path.dirname(os.path.abspath(__file__))
    with tempfile.TemporaryDirectory() as td:
        inp = os.path.join(td, "io.pkl")
        with open(inp, "wb") as f:
            pickle.dump({"x": x, "y": y}, f)
        code = (
            "import pickle, sys; sys.path.insert(0, %r); import kernel as km; "
            "d = pickle.load(open(%r, 'rb')); "
            "r = km.kernel(d['x'], d['y']); "
            "pickle.dump(r, open(%r, 'wb'))"
            % (kdir, inp, inp + ".out")
        )
        for attempt in range(2):
            p = subprocess.run(
                [sys.executable, "-c", code], capture_output=True, timeout=1800
            )
            if p.returncode == 0 and os.path.exists(inp + ".out"):
                with open(inp + ".out", "rb") as f:
                    return pickle.load(f)
    raise last


def kernel(x, y, _trace=False, _trace_kw=None):
    x = np.asarray(x, np.float32)
    y = np.asarray(y, np.float32)
    res = _run_with_retries(x, y, trace=_trace, _trace_kw=_trace_kw)
    if isinstance(res, np.floating | np.ndarray):
        return res

    c_xx = 0.0
    c_xy = 0.0
    for c in range(NCORES):
        acc = np.asarray(res.results[c]["ACC"], np.float64)  # (128, 2*SPT)
        c_xy += acc[:, 0::2].sum()
        c_xx += acc[:, 1::2].sum()
    out = (c_xx - 512.0 * 254.0 - c_xy) / (B * (B - 1)) / 254.0
    if _trace:
        kernel.last_results = res
    return np.float32(out)
